# revision 7
# baseline (speedup 1.0000x reference)
"""Distributed contrastive loss kernel for 8 Trainium2 NeuronCores.

loss = mean_i( logsumexp_j(f1n_i . f2n_j / T) - (f1n_i . f2n_i) / T )
with f1n/f2n the L2-row-normalized feature matrices, N=16384, D=512.

Sharding: f1 rows are sharded 8 ways (2048 rows/core); each core computes
its [2048, 16384] slab of logits entirely on-chip (fused matmul -> exp ->
row-sum, logits never hit HBM) and reduces to one partial-loss scalar.
The host sums the 8 partials and divides by N.
"""

import os
from functools import lru_cache

import numpy as np

import concourse.bass as bass
import concourse.mybir as mybir
import concourse.tile as tile
from concourse.bass_utils import run_bass_kernel_spmd
from concourse.masks import make_identity

# Problem shape (hardcoded per contest rules).
N = 16384
D = 512
N_CORES = 8
M_LOCAL = N // N_CORES  # 2048 rows of f1 per core
TEMP = 0.07

P = 128                 # SBUF partitions
DC = D // P             # 4 contraction chunks
MT = M_LOCAL // P       # 16 m-tiles per core
NCH = 512               # matmul moving-operand free dim (one PSUM bank)
NCHUNKS = N // NCH      # 32 n-chunks
NGRP = 8                # f2nT column groups (pipelining granularity)
NGW = N // NGRP         # 2048 columns per group
F32 = mybir.dt.float32
BF16 = mybir.dt.bfloat16

# Module-level stash for the last run's profile (read by test.py).
LAST_EXEC_TIME_NS = None


def _install_ntff_hook():
    """Provide antenv.axon_hooks (missing from this image) so that
    run_bass_kernel_spmd(trace=True) can capture NTFF profiles via the
    axon PJRT .so. Mirrors trn_agent_boot.trn_boot._ntff_profile_via_ctypes."""
    import contextlib
    import ctypes
    import sys
    import types

    try:
        import antenv.axon_hooks  # noqa: F401

        return
    except ImportError:
        pass

    so_path = "/opt/axon/libaxon_pjrt.so"
    hook = None
    try:
        lib = ctypes.CDLL(so_path)
        if hasattr(lib, "axon_start_nrt_profile"):
            lib.axon_start_nrt_profile.argtypes = [
                ctypes.POINTER(ctypes.c_int64),
                ctypes.c_size_t,
            ]
            lib.axon_start_nrt_profile.restype = ctypes.c_int64
            lib.axon_stop_nrt_profile.argtypes = [ctypes.c_char_p]
            lib.axon_stop_nrt_profile.restype = ctypes.c_int64

            @contextlib.contextmanager
            def _hook(output_dir, device_ids):
                import jax

                jax.devices()
                if device_ids:
                    ids = (ctypes.c_int64 * len(device_ids))(*device_ids)
                    rc = lib.axon_start_nrt_profile(ids, len(device_ids))
                else:
                    rc = lib.axon_start_nrt_profile(None, 0)
                if rc != 0:
                    raise RuntimeError(f"axon_start_nrt_profile rc={rc}")
                try:
                    yield
                finally:
                    n = lib.axon_stop_nrt_profile(str(output_dir).encode())
                    print(f"profile: {n} file(s) written to {output_dir}", file=sys.stderr)

            hook = _hook
    except OSError:
        pass

    import antenv

    mod = types.ModuleType("antenv.axon_hooks")
    _state = {"hook": hook}
    mod.set_axon_ntff_profile_hook = lambda h: _state.__setitem__("hook", h)
    mod.get_axon_ntff_profile_hook = lambda: _state["hook"]
    sys.modules["antenv.axon_hooks"] = mod
    antenv.axon_hooks = mod

    # Artifact upload needs external storage creds; make it a no-op.
    import concourse.bass_utils as _bu

    _bu.upload_artifacts = lambda tmpdir: f"local:{tmpdir}"


def _build_bass():
    nc = bass.Bass(num_devices=N_CORES, debug=False)

    f1s = nc.dram_tensor("f1s", [M_LOCAL, D], F32, kind="ExternalInput")
    f2o = nc.dram_tensor("f2o", [M_LOCAL, D], F32, kind="ExternalInput")
    f2 = nc.dram_tensor("f2", [N, D], F32, kind="ExternalInput")
    out = nc.dram_tensor("out", [1, 1], F32, kind="ExternalOutput")

    from contextlib import ExitStack

    with tile.TileContext(nc) as tc, ExitStack() as ctx:
        consts = ctx.enter_context(tc.tile_pool(name="consts", bufs=1))
        resident = ctx.enter_context(tc.tile_pool(name="resident", bufs=1))
        loads = ctx.enter_context(tc.tile_pool(name="loads", bufs=4))
        work = ctx.enter_context(tc.tile_pool(name="work", bufs=3))
        stats = ctx.enter_context(tc.tile_pool(name="stats", bufs=4))
        psum_mm = ctx.enter_context(tc.tile_pool(name="psum_mm", bufs=4, space="PSUM"))
        psum_tp = ctx.enter_context(tc.tile_pool(name="psum_tp", bufs=3, space="PSUM"))

        identity = consts.tile([P, P], BF16)
        make_identity(nc, identity)
        ones_col = consts.tile([P, 1], F32)
        nc.vector.memset(ones_col, 1.0)

        # Resident transposed, normalized, bf16 operands (d on partitions).
        f1nT = [resident.tile([P, M_LOCAL], BF16, name=f"f1nT{c}") for c in range(DC)]
        f2nT = [
            [resident.tile([P, NGW], BF16, name=f"f2nT{c}_{g}") for g in range(NGRP)]
            for c in range(DC)
        ]
        diag = resident.tile([P, MT], F32, name="diag")          # raw f1n.f2n dots
        rowsums = [
            resident.tile([P, NCHUNKS], F32, name=f"rs{mt}") for mt in range(MT)
        ]
        losses = resident.tile([P, MT], F32, name="losses")

        def normalize_tile(x, tag):
            """x: [P, D] f32 SBUF tile -> (xn_bf16, inv_norm[P,1] f32)."""
            sq = work.tile([P, D], F32, tag="sq", bufs=2, name=f"sq_{tag}")
            ss = stats.tile([P, 1], F32, tag="ss", name=f"ss_{tag}")
            nc.scalar.activation(
                sq, x, mybir.ActivationFunctionType.Square, accum_out=ss
            )
            nrm = stats.tile([P, 1], F32, tag="nrm", name=f"nrm_{tag}")
            nc.scalar.activation(nrm, ss, mybir.ActivationFunctionType.Sqrt)
            inv = stats.tile([P, 1], F32, tag="inv", name=f"inv_{tag}")
            nc.vector.reciprocal(inv, nrm)
            xn = work.tile([P, D], BF16, tag="xn", name=f"xn_{tag}")
            nc.vector.tensor_scalar_mul(xn, x, inv)
            return xn, inv

        # ---- Phase 1: own shards -> f1nT, diag -------------------------------
        for t in range(MT):
            x1 = loads.tile([P, D], F32, tag="x", name="x1")
            nc.sync.dma_start(out=x1, in_=f1s[t * P : (t + 1) * P, :])
            x2 = loads.tile([P, D], F32, tag="x", name="x2")
            nc.sync.dma_start(out=x2, in_=f2o[t * P : (t + 1) * P, :])

            x1n, inv1 = normalize_tile(x1, f"f1_{t}")

            # inv2 for the own f2 rows (no cast/transpose needed).
            sq2 = work.tile([P, D], F32, tag="sq", bufs=2, name="sq2")
            ss2 = stats.tile([P, 1], F32, tag="ss", name="ss2")
            nc.scalar.activation(
                sq2, x2, mybir.ActivationFunctionType.Square, accum_out=ss2
            )
            nrm2 = stats.tile([P, 1], F32, tag="nrm", name="nrm2")
            nc.scalar.activation(nrm2, ss2, mybir.ActivationFunctionType.Sqrt)
            inv2 = stats.tile([P, 1], F32, tag="inv", name="inv2")
            nc.vector.reciprocal(inv2, nrm2)

            # Raw per-row dot f1.f2 (f32), then scale by inv1*inv2.
            prod = work.tile([P, D], F32, tag="prod", bufs=2, name="prod")
            dotr = stats.tile([P, 1], F32, tag="dot", name="dotr")
            nc.vector.tensor_tensor(prod, x1, x2, mybir.AluOpType.mult)
            nc.vector.reduce_sum(dotr, prod, axis=mybir.AxisListType.X)
            nc.vector.tensor_scalar(
                out=diag[:, t : t + 1],
                in0=dotr,
                scalar1=inv1,
                scalar2=inv2,
                op0=mybir.AluOpType.mult,
                op1=mybir.AluOpType.mult,
            )

            for c in range(DC):
                tp = psum_tp.tile([P, P], BF16, tag="tp", name="tp1")
                nc.tensor.transpose(tp, x1n[:, c * P : (c + 1) * P], identity)
                nc.any.tensor_copy(f1nT[c][:, t * P : (t + 1) * P], tp)

        # ---- Phase 2: full f2 -> f2nT ---------------------------------------
        for g in range(N // P):  # 128 natural tiles
            x = loads.tile([P, D], F32, tag="x", name="x2f")
            nc.sync.dma_start(out=x, in_=f2[g * P : (g + 1) * P, :])
            xn, _ = normalize_tile(x, f"f2_{g}")
            grp, off = divmod(g * P, NGW)
            for c in range(DC):
                tp = psum_tp.tile([P, P], BF16, tag="tp", name="tp2")
                nc.tensor.transpose(tp, xn[:, c * P : (c + 1) * P], identity)
                nc.any.tensor_copy(f2nT[c][grp][:, off : off + P], tp)

        # ---- Phase 3: fused logits -> exp -> row-sums ------------------------
        inv_temp = 1.0 / TEMP
        for nch in range(NCHUNKS):
            grp, off = divmod(nch * NCH, NGW)
            for mt in range(MT):
                ps = psum_mm.tile([P, NCH], F32, tag="ps", name="ps")
                for c in range(DC):
                    nc.tensor.matmul(
                        ps,
                        lhsT=f1nT[c][:, mt * P : (mt + 1) * P],
                        rhs=f2nT[c][grp][:, off : off + NCH],
                        start=(c == 0),
                        stop=(c == DC - 1),
                    )
                ex = work.tile([P, NCH], BF16, tag="ex", bufs=2, name="ex")
                nc.scalar.activation(
                    ex,
                    ps,
                    mybir.ActivationFunctionType.Exp,
                    scale=inv_temp,
                    accum_out=rowsums[mt][:, nch : nch + 1],
                )

        # ---- Phase 4: logsumexp, subtract diag, reduce -----------------------
        for mt in range(MT):
            s = stats.tile([P, 1], F32, tag="s", name="s")
            nc.vector.reduce_sum(s, rowsums[mt], axis=mybir.AxisListType.X)
            lse = stats.tile([P, 1], F32, tag="lse", name="lse")
            nc.scalar.activation(lse, s, mybir.ActivationFunctionType.Ln)
            # losses[:, mt] = lse - diag/T = (diag * -1/T) + lse
            nc.vector.scalar_tensor_tensor(
                out=losses[:, mt : mt + 1],
                in0=diag[:, mt : mt + 1],
                scalar=-inv_temp,
                in1=lse,
                op0=mybir.AluOpType.mult,
                op1=mybir.AluOpType.add,
            )

        loss_col = stats.tile([P, 1], F32, tag="lc", name="loss_col")
        nc.vector.reduce_sum(loss_col, losses, axis=mybir.AxisListType.X)
        fin = psum_mm.tile([1, 1], F32, tag="ps", name="fin")
        nc.tensor.matmul(fin, lhsT=loss_col, rhs=ones_col, start=True, stop=True)
        res = stats.tile([1, 1], F32, tag="res", name="res")
        nc.any.tensor_copy(res, fin)
        nc.sync.dma_start(out=out[:, :], in_=res)

    return nc


_WAIT_EXEMPT = ("InstCall",)


def _legalize_sync_waits(nc, limit=1):
    """Walrus codegen rejects instructions carrying more than ~1 embedded
    semaphore wait ("Too many sync wait commands"). Move excess waits onto
    injected same-engine NoOps (one wait each) ahead of the instruction —
    semantically identical (the engine blocks on the NoOps first)."""
    n_split = 0
    for b in nc.m.functions[0].blocks:
        insts = b.instructions
        out = []
        changed = False
        for ins in insts:
            si = ins.sync_info
            tname = type(ins).__name__
            if (
                si is not None
                and len(si.on_wait) > limit
                and tname not in _WAIT_EXEMPT
            ):
                waits = list(si.on_wait)
                keep, excess = waits[:limit], waits[limit:]
                for j, w in enumerate(excess):
                    noop = mybir.InstNoOp(name=f"{ins.name}-ws{j}", ins=[], outs=[])
                    noop.engine = ins.engine
                    noop.sync_info = mybir.SyncInfo(on_wait=[w], on_update=[])
                    out.append(noop)
                ins.sync_info = mybir.SyncInfo(
                    on_wait=keep, on_update=list(si.on_update)
                )
                n_split += 1
                changed = True
            out.append(ins)
        if changed:
            b.instructions = out
    return n_split


@lru_cache(maxsize=1)
def _get_nc():
    nc = _build_bass()
    _legalize_sync_waits(nc)
    return nc


def kernel(features1, features2):
    global LAST_EXEC_TIME_NS
    f1 = np.ascontiguousarray(np.asarray(features1, dtype=np.float32))
    f2 = np.ascontiguousarray(np.asarray(features2, dtype=np.float32))
    assert f1.shape == (N, D) and f2.shape == (N, D)

    in_maps = []
    for i in range(N_CORES):
        sl = slice(i * M_LOCAL, (i + 1) * M_LOCAL)
        in_maps.append(
            {
                "f1s": np.ascontiguousarray(f1[sl]),
                "f2o": np.ascontiguousarray(f2[sl]),
                "f2": f2,
            }
        )

    nc = _get_nc()
    trace = bool(int(os.environ.get("KERNEL_TRACE", "0")))
    if trace:
        _install_ntff_hook()
    tmpdir = os.environ.get("KERNEL_TRACE_DIR") or None
    r = run_bass_kernel_spmd(
        nc, in_maps, list(range(N_CORES)), trace=trace, tmpdir=tmpdir
    )
    LAST_EXEC_TIME_NS = r.exec_time_ns

    total = sum(float(r.results[i]["out"][0, 0]) for i in range(N_CORES))
    return np.float32(total / N)
